# revision 16
# baseline (speedup 1.0000x reference)
"""Multi-head attention kernel for 8 TRN2 NeuronCores.

Problem: x[4,2048,1024] -> qkv proj (w_qkv[1024,3072]) -> 16-head attention
(dim_head=64, scale=1024**-0.5) -> out proj (w_out[1024,1024] + b_out).

Sharding: core c in 0..7 handles batch b=c//2, head-group g=c%2 (8 heads).
Each core computes a partial output y_partial = attn_out_g @ w_out[rows_g];
host sums the pair (the tensor-parallel all-reduce, done at unshard time).

Layout strategy (zero on-chip transposes):
  - host supplies xT = x[b].T                     [1024, 2048] fp16
  - qkT = (x @ w_qk).T computed directly:  lhsT=w chunk, rhs=xT  -> [c, i]
  - V   = x @ w_v computed normally:       lhsT=xT chunk, rhs=wv -> [i, c]
  - S^T = k_h @ q_h^T per head:            lhsT=kT slice, rhs=qT slice
          -> [keys, q]; heads processed in pairs, the even head in array
          row-group 0-63 and the odd head in 64-127, so their K=64
          matmuls run concurrently in the PE array
  - P   = exp(S^T * scale)  (no max subtraction: |S*scale| < ~1)
  - O^T|s = [v_h | 1]^T @ P : lhsT=v[128,65] (ones col), rhs=P -> [65, q]
          row 64 is the softmax denominator s
  - normalize off critical path: 1/s row bounced through DRAM and
    broadcast-DMA'd across partitions (no PSUM, no PE involvement)
  - y = sum_h (O_h^T).T @ w_out_h : lhsT=otn[64,128], rhs=wo -> [i, e]
qkv matmuls for later head-pairs are interleaved one-per-kc into the
attention loop as PE filler under the ScalarE-exp-bound steady state.
All matmul inputs fp16, PSUM accumulation fp32, output fp32.
"""

import numpy as np

B, N, D = 4, 2048, 1024
HEADS, DH = 16, 64
HP = HEADS // 2          # heads per core
GDIM = HP * DH           # 512 columns per head-group
SCALE = float(D) ** -0.5
NCORES = 8

_CACHE = {}


def _build():
    from contextlib import ExitStack

    import concourse.bass as bass
    import concourse.tile as tile
    from concourse import bacc, mybir

    F16 = mybir.dt.float16
    F32 = mybir.dt.float32
    EXP = mybir.ActivationFunctionType.Exp

    nc = bacc.Bacc(None, target_bir_lowering=False)

    xT_d = nc.declare_dram_parameter("xT", [D, N], F16, isOutput=False)
    wqk_d = nc.declare_dram_parameter("wqk", [D, 2 * GDIM], F16, isOutput=False)
    wv_d = nc.declare_dram_parameter("wv", [D, GDIM], F16, isOutput=False)
    wo_d = nc.declare_dram_parameter("wo", [HP, DH, D], F16, isOutput=False)
    bias_d = nc.declare_dram_parameter("bias", [D], F32, isOutput=False)
    out_d = nc.declare_dram_parameter("out", [N, D], F32, isOutput=True)

    with tile.TileContext(nc) as tc, ExitStack() as ctx:
        persist = ctx.enter_context(tc.tile_pool(name="persist", bufs=1))
        ptp = ctx.enter_context(tc.tile_pool(name="ptp", bufs=6))
        rawp = ctx.enter_context(tc.tile_pool(name="rawp", bufs=6))
        tiny = ctx.enter_context(tc.tile_pool(name="tiny", bufs=4))
        ypool = ctx.enter_context(tc.tile_pool(name="ypool", bufs=2))
        dramp = ctx.enter_context(tc.tile_pool(name="dramp", bufs=4,
                                               space="DRAM"))
        # PSUM budget (8 banks): stq [128,1024] x2 bufs = 4, ot x4 = 4... see
        # tags: "stq" 2-bank tiles bufs=2, "ot0..3" 1 bank each, "qf" 1 bank
        mm = ctx.enter_context(tc.tile_pool(name="mm", bufs=2, space="PSUM"))
        acc = ctx.enter_context(tc.tile_pool(name="acc", bufs=1, space="PSUM"))

        # ---- persistent SBUF tiles -------------------------------------
        xT = [persist.tile([128, N], F16, name=f"xT{e}", tag=f"xT{e}")
              for e in range(8)]
        wqk = [persist.tile([128, 2 * GDIM], F16, name=f"wqk{e}", tag=f"wqk{e}")
               for e in range(8)]
        wv = [persist.tile([128, GDIM], F16, name=f"wv{e}", tag=f"wv{e}")
              for e in range(8)]
        wo = [persist.tile([DH, D], F16, name=f"wo{h}", tag=f"wo{h}")
              for h in range(HP)]
        bias = persist.tile([128, D], F32, tag="bias")
        qkT = [persist.tile([128, N], F16, name=f"qkT{c}", tag=f"qkT{c}")
               for c in range(8)]
        vt = [persist.tile([128, HP, DH + 1], F16, name=f"v{kc}", tag=f"v{kc}")
              for kc in range(16)]
        otn = [persist.tile([DH, N], F16, name=f"otn{h}", tag=f"otn{h}")
               for h in range(HP)]

        for e in range(8):
            nc.sync.dma_start(out=xT[e], in_=xT_d[e * 128:(e + 1) * 128, :])
            nc.sync.dma_start(out=wqk[e], in_=wqk_d[e * 128:(e + 1) * 128, :])
            nc.sync.dma_start(out=wv[e], in_=wv_d[e * 128:(e + 1) * 128, :])
        for h in range(HP):
            nc.sync.dma_start(out=wo[h], in_=wo_d[h])
        bias_ap = bias_d[:]
        nc.sync.dma_start(
            out=bias,
            in_=bass.AP(tensor=bias_ap.tensor, offset=bias_ap.offset,
                        ap=[[0, 128]] + list(bias_ap.ap)),
        )
        for kc in range(16):
            nc.vector.memset(vt[kc][:, :, DH:DH + 1], 1.0)

        def qkv_chain(c, ih):
            """One [128,1024] qkT chunk: chunk c, query half ih."""
            ps = mm.tile([128, 1024], F32, name="stq", tag="stq")
            for e in range(8):
                w_sl = wqk[e][:, c * 128:(c + 1) * 128]
                yield nc.tensor.matmul(
                    ps[:, 0:512], lhsT=w_sl,
                    rhs=xT[e][:, ih * 1024:ih * 1024 + 512],
                    start=(e == 0), stop=(e == 7))
                yield nc.tensor.matmul(
                    ps[:, 512:1024], lhsT=w_sl,
                    rhs=xT[e][:, ih * 1024 + 512:(ih + 1) * 1024],
                    start=(e == 0), stop=(e == 7))
            yield nc.vector.tensor_copy(
                qkT[c][:, ih * 1024:(ih + 1) * 1024], ps)

        def v_chain(ih):
            """Two key-tiles of V via one [128,1024] psum tile."""
            ps = mm.tile([128, 1024], F32, name="stq", tag="stq")
            for e in range(8):
                yield nc.tensor.matmul(
                    ps[:, 0:512],
                    lhsT=xT[e][:, (2 * ih) * 128:(2 * ih + 1) * 128],
                    rhs=wv[e], start=(e == 0), stop=(e == 7))
                yield nc.tensor.matmul(
                    ps[:, 512:1024],
                    lhsT=xT[e][:, (2 * ih + 1) * 128:(2 * ih + 2) * 128],
                    rhs=wv[e], start=(e == 0), stop=(e == 7))
            for j in range(2):
                yield nc.vector.tensor_copy(
                    vt[2 * ih + j][:, :, 0:DH],
                    ps[:, j * 512:(j + 1) * 512].rearrange(
                        "p (h d) -> p h d", h=HP))

        # ---- prelude: full qkv + V projections --------------------------
        for c in range(8):
            for ih in range(2):
                for g in qkv_chain(c, ih):
                    pass
        for ih in range(8):
            for g in v_chain(ih):
                pass

        # ---- attention: head pairs, packed ST row-groups ---------------
        for t in range(4):
            hA, hB = 2 * t, 2 * t + 1
            qch, kch = t, 4 + t
            for qh in range(2):
                ot = {}
                for j, hh in ((0, hA), (1, hA), (2, hB), (3, hB)):
                    ot[j] = acc.tile([65, 512], F32,
                                     name=f"ot{t}_{qh}_{j}", tag=f"ot{j}")
                def emit_ot(kc, ptA, ptB):
                    vA = vt[kc][:, hA, :]
                    vB = vt[kc][:, hB, :]
                    st, sp = (kc == 0), (kc == 15)
                    nc.tensor.matmul(ot[0], lhsT=vA, rhs=ptA[:, 0:512],
                                     start=st, stop=sp, skip_group_check=True)
                    nc.tensor.matmul(ot[1], lhsT=vA, rhs=ptA[:, 512:1024],
                                     start=st, stop=sp, skip_group_check=True)
                    nc.tensor.matmul(ot[2], lhsT=vB, rhs=ptB[:, 0:512],
                                     start=st, stop=sp, skip_group_check=True)
                    nc.tensor.matmul(ot[3], lhsT=vB, rhs=ptB[:, 512:1024],
                                     start=st, stop=sp, skip_group_check=True)

                pt_hist = []
                for kc in range(16):
                    kA = qkT[kch][0:64, kc * 128:(kc + 1) * 128]
                    kB = qkT[kch][64:128, kc * 128:(kc + 1) * 128]
                    stqA = mm.tile([128, 1024], F32, name="stq", tag="stq")
                    nc.tensor.matmul(
                        stqA[:, 0:512], lhsT=kA,
                        rhs=qkT[qch][0:64, qh * 1024:qh * 1024 + 512],
                        start=True, stop=True)
                    nc.tensor.matmul(
                        stqA[:, 512:1024], lhsT=kA,
                        rhs=qkT[qch][0:64, qh * 1024 + 512:(qh + 1) * 1024],
                        start=True, stop=True)
                    stqB = mm.tile([128, 1024], F32, name="stq", tag="stq")
                    nc.tensor.matmul(
                        stqB[:, 0:512], lhsT=kB,
                        rhs=qkT[qch][64:128, qh * 1024:qh * 1024 + 512],
                        start=True, stop=True)
                    nc.tensor.matmul(
                        stqB[:, 512:1024], lhsT=kB,
                        rhs=qkT[qch][64:128, qh * 1024 + 512:(qh + 1) * 1024],
                        start=True, stop=True)
                    ptA = ptp.tile([128, 1024], F16, name="pt", tag="pt")
                    nc.scalar.activation(ptA, stqA, EXP, scale=SCALE)
                    ptB = ptp.tile([128, 1024], F16, name="pt", tag="pt")
                    nc.scalar.activation(ptB, stqB, EXP, scale=SCALE)
                    pt_hist.append((kc, ptA, ptB))
                    if len(pt_hist) > 2:
                        k2, a2, b2 = pt_hist.pop(0)
                        emit_ot(k2, a2, b2)
                for k2, a2, b2 in pt_hist:
                    emit_ot(k2, a2, b2)
                # normalize (off critical path): raw copies FIRST so all
                # ot banks free quickly; recips/dmas/muls trail behind.
                js = ((0, hA), (1, hA), (2, hB), (3, hB))
                raws, rcs, bcs = {}, {}, {}
                for j, hh in js:
                    raw = rawp.tile([65, 512], F16, name="raw", tag="raw")
                    nc.vector.tensor_copy(raw, ot[j])
                    raws[j] = raw
                    s32 = tiny.tile([65, 512], F32, name="s32", tag="s32",
                                    bufs=2)
                    nc.vector.tensor_copy(s32[64:65, :], ot[j][64:65, :])
                    rc = tiny.tile([65, 512], F32, name="rc", tag="rc",
                                   bufs=2)
                    nc.vector.reciprocal(rc[64:65, :], s32[64:65, :])
                    rcs[j] = rc
                    dsc = dramp.tile([512], F32, name="dsc", tag="dsc")
                    nc.sync.dma_start(out=dsc, in_=rc[64:65, :])
                    bc = tiny.tile([64, 512], F32, name="bc", tag="bc")
                    dap = dsc[:]
                    nc.sync.dma_start(
                        out=bc,
                        in_=bass.AP(tensor=dap.tensor, offset=dap.offset,
                                    ap=[[0, 64]] + list(dap.ap)))
                    bcs[j] = bc
                for j, hh in js:
                    qc = 2 * qh + (j % 2)
                    nc.vector.tensor_mul(
                        otn[hh][:, qc * 512:(qc + 1) * 512],
                        raws[j][0:64, :], bcs[j])

        # ---- output projection: 32 narrow chains, 6 psum slots ---------
        ptags = ["stq", "stq", "ot0", "ot1", "ot2", "ot3"]
        ppools = [mm, mm, acc, acc, acc, acc]
        ci = 0
        for it in range(16):
            for half in range(2):
                tag = ptags[ci % 6]
                ps = ppools[ci % 6].tile([128, 512], F32,
                                         name=f"pj{ci}", tag=tag)
                ci += 1
                e0 = half * 512
                for h in range(HP):
                    nc.tensor.matmul(
                        ps, lhsT=otn[h][:, it * 128:(it + 1) * 128],
                        rhs=wo[h][:, e0:e0 + 512],
                        start=(h == 0), stop=(h == 7))
                yt = ypool.tile([128, 512], F32, name="yt", tag="yt",
                                bufs=4)
                nc.vector.tensor_add(yt, ps, bias[:, e0:e0 + 512])
                nc.sync.dma_start(
                    out=out_d[it * 128:(it + 1) * 128, e0:e0 + 512], in_=yt)

    nc.compile()
    return nc


def _in_maps(x, w_qkv, w_out, b_out):
    x = np.asarray(x, dtype=np.float32)
    w_qkv = np.asarray(w_qkv, dtype=np.float32)
    w_out = np.asarray(w_out, dtype=np.float32)
    b_out = np.asarray(b_out, dtype=np.float32)
    maps = []
    for c in range(NCORES):
        b, g = c // 2, c % 2
        qcols = w_qkv[:, g * GDIM:(g + 1) * GDIM]
        kcols = w_qkv[:, D + g * GDIM:D + (g + 1) * GDIM]
        vcols = w_qkv[:, 2 * D + g * GDIM:2 * D + (g + 1) * GDIM]
        maps.append({
            "xT": np.ascontiguousarray(x[b].T).astype(np.float16),
            "wqk": np.concatenate([qcols, kcols], axis=1).astype(np.float16),
            "wv": np.ascontiguousarray(vcols).astype(np.float16),
            "wo": np.ascontiguousarray(
                w_out[g * GDIM:(g + 1) * GDIM, :].reshape(HP, DH, D)
            ).astype(np.float16),
            "bias": (b_out if g == 0 else np.zeros_like(b_out)),
        })
    return maps


def kernel(x, w_qkv, w_out, b_out):
    from concourse.bass_utils import run_bass_kernel_spmd

    if "nc" not in _CACHE:
        _CACHE["nc"] = _build()
    nc = _CACHE["nc"]
    maps = _in_maps(x, w_qkv, w_out, b_out)
    res = run_bass_kernel_spmd(nc, maps, core_ids=list(range(NCORES)))
    outs = res.results
    y = np.empty((B, N, D), dtype=np.float32)
    for b in range(B):
        y[b] = outs[2 * b]["out"] + outs[2 * b + 1]["out"]
    return y


# revision 17
# speedup vs baseline: 1.1764x; 1.1764x over previous
"""Multi-head attention kernel for 8 TRN2 NeuronCores.

Problem: x[4,2048,1024] -> qkv proj (w_qkv[1024,3072]) -> 16-head attention
(dim_head=64, scale=1024**-0.5) -> out proj (w_out[1024,1024] + b_out).

Sharding: core c in 0..7 handles batch b=c//2, head-group g=c%2 (8 heads).
Each core computes a partial output y_partial = attn_out_g @ w_out[rows_g];
host sums the pair (the tensor-parallel all-reduce, done at unshard time).

Layout strategy (zero on-chip transposes):
  - host supplies xT = x[b].T                     [1024, 2048] fp16
  - qkT = (x @ w_qk).T computed directly:  lhsT=w chunk, rhs=xT  -> [c, i]
  - V   = x @ w_v computed normally:       lhsT=xT chunk, rhs=wv -> [i, c]
  - S^T = k_h @ q_h^T per head:            lhsT=kT slice, rhs=qT slice
          -> [keys, q]; heads processed in pairs, the even head in array
          row-group 0-63 and the odd head in 64-127, so their K=64
          matmuls run concurrently in the PE array
  - P   = exp(S^T * scale)  (no max subtraction: |S*scale| < ~1)
  - O^T|s = [v_h | 1]^T @ P : lhsT=v[128,65] (ones col), rhs=P -> [65, q]
          row 64 is the softmax denominator s
  - normalize off critical path: 1/s row bounced through DRAM and
    broadcast-DMA'd across partitions (no PSUM, no PE involvement)
  - y = sum_h (O_h^T).T @ w_out_h : lhsT=otn[64,128], rhs=wo -> [i, e]
qkv matmuls for later head-pairs are interleaved one-per-kc into the
attention loop as PE filler under the ScalarE-exp-bound steady state.
All matmul inputs fp16, PSUM accumulation fp32, output fp32.
"""

import numpy as np

B, N, D = 4, 2048, 1024
HEADS, DH = 16, 64
HP = HEADS // 2          # heads per core
GDIM = HP * DH           # 512 columns per head-group
SCALE = float(D) ** -0.5
NCORES = 8

_CACHE = {}


def _build():
    from contextlib import ExitStack

    import concourse.bass as bass
    import concourse.tile as tile
    from concourse import bacc, mybir

    F16 = mybir.dt.float16
    F32 = mybir.dt.float32
    EXP = mybir.ActivationFunctionType.Exp

    nc = bacc.Bacc(None, target_bir_lowering=False)

    xT_d = nc.declare_dram_parameter("xT", [D, N], F16, isOutput=False)
    wqk_d = nc.declare_dram_parameter("wqk", [D, 2 * GDIM], F16, isOutput=False)
    wv_d = nc.declare_dram_parameter("wv", [D, GDIM], F16, isOutput=False)
    wo_d = nc.declare_dram_parameter("wo", [HP, DH, D], F16, isOutput=False)
    bias_d = nc.declare_dram_parameter("bias", [D], F32, isOutput=False)
    out_d = nc.declare_dram_parameter("out", [N, D], F32, isOutput=True)

    with tile.TileContext(nc) as tc, ExitStack() as ctx:
        persist = ctx.enter_context(tc.tile_pool(name="persist", bufs=1))
        ptp = ctx.enter_context(tc.tile_pool(name="ptp", bufs=6))
        rawp = ctx.enter_context(tc.tile_pool(name="rawp", bufs=6))
        tiny = ctx.enter_context(tc.tile_pool(name="tiny", bufs=4))
        ypool = ctx.enter_context(tc.tile_pool(name="ypool", bufs=2))
        dramp = ctx.enter_context(tc.tile_pool(name="dramp", bufs=4,
                                               space="DRAM"))
        # PSUM budget (8 banks): stq [128,1024] x2 bufs = 4, ot x4 = 4... see
        # tags: "stq" 2-bank tiles bufs=2, "ot0..3" 1 bank each, "qf" 1 bank
        mm = ctx.enter_context(tc.tile_pool(name="mm", bufs=2, space="PSUM"))
        acc = ctx.enter_context(tc.tile_pool(name="acc", bufs=1, space="PSUM"))

        # ---- persistent SBUF tiles -------------------------------------
        xT = [persist.tile([128, N], F16, name=f"xT{e}", tag=f"xT{e}")
              for e in range(8)]
        wqk = [persist.tile([128, 2 * GDIM], F16, name=f"wqk{e}", tag=f"wqk{e}")
               for e in range(8)]
        wv = [persist.tile([128, GDIM], F16, name=f"wv{e}", tag=f"wv{e}")
              for e in range(8)]
        wo = [persist.tile([DH, D], F16, name=f"wo{h}", tag=f"wo{h}")
              for h in range(HP)]
        bias = persist.tile([128, D], F32, tag="bias")
        qkT = [persist.tile([128, N], F16, name=f"qkT{c}", tag=f"qkT{c}")
               for c in range(8)]
        vt = [persist.tile([128, HP, DH + 1], F16, name=f"v{kc}", tag=f"v{kc}")
              for kc in range(16)]
        otn = [persist.tile([DH, N], F16, name=f"otn{h}", tag=f"otn{h}")
               for h in range(HP)]

        for e in range(8):
            nc.sync.dma_start(out=xT[e], in_=xT_d[e * 128:(e + 1) * 128, :])
            nc.sync.dma_start(out=wqk[e], in_=wqk_d[e * 128:(e + 1) * 128, :])
            nc.sync.dma_start(out=wv[e], in_=wv_d[e * 128:(e + 1) * 128, :])
        for h in range(HP):
            nc.sync.dma_start(out=wo[h], in_=wo_d[h])
        bias_ap = bias_d[:]
        nc.sync.dma_start(
            out=bias,
            in_=bass.AP(tensor=bias_ap.tensor, offset=bias_ap.offset,
                        ap=[[0, 128]] + list(bias_ap.ap)),
        )
        for kc in range(16):
            nc.vector.memset(vt[kc][:, :, DH:DH + 1], 1.0)

        def qkv_chain(c, ih):
            """One [128,1024] qkT chunk: chunk c, query half ih."""
            ps = mm.tile([128, 1024], F32, name="stq", tag="stq")
            for e in range(8):
                w_sl = wqk[e][:, c * 128:(c + 1) * 128]
                yield nc.tensor.matmul(
                    ps[:, 0:512], lhsT=w_sl,
                    rhs=xT[e][:, ih * 1024:ih * 1024 + 512],
                    start=(e == 0), stop=(e == 7))
                yield nc.tensor.matmul(
                    ps[:, 512:1024], lhsT=w_sl,
                    rhs=xT[e][:, ih * 1024 + 512:(ih + 1) * 1024],
                    start=(e == 0), stop=(e == 7))
            yield nc.vector.tensor_copy(
                qkT[c][:, ih * 1024:(ih + 1) * 1024], ps)

        def v_chain(ih):
            """Two key-tiles of V via one [128,1024] psum tile."""
            ps = mm.tile([128, 1024], F32, name="stq", tag="stq")
            for e in range(8):
                yield nc.tensor.matmul(
                    ps[:, 0:512],
                    lhsT=xT[e][:, (2 * ih) * 128:(2 * ih + 1) * 128],
                    rhs=wv[e], start=(e == 0), stop=(e == 7))
                yield nc.tensor.matmul(
                    ps[:, 512:1024],
                    lhsT=xT[e][:, (2 * ih + 1) * 128:(2 * ih + 2) * 128],
                    rhs=wv[e], start=(e == 0), stop=(e == 7))
            for j in range(2):
                yield nc.vector.tensor_copy(
                    vt[2 * ih + j][:, :, 0:DH],
                    ps[:, j * 512:(j + 1) * 512].rearrange(
                        "p (h d) -> p h d", h=HP))

        # ---- prelude: full qkv + V projections --------------------------
        for c in range(8):
            for ih in range(2):
                for g in qkv_chain(c, ih):
                    pass
        for ih in range(8):
            for g in v_chain(ih):
                pass

        # ---- attention: head pairs, packed ST row-groups ---------------
        for t in range(4):
            hA, hB = 2 * t, 2 * t + 1
            qch, kch = t, 4 + t
            for qh in range(2):
                ot = {}
                for j, hh in ((0, hA), (1, hA), (2, hB), (3, hB)):
                    ot[j] = acc.tile([65, 512], F32,
                                     name=f"ot{t}_{qh}_{j}", tag=f"ot{j}")
                def emit_ot(kc, ptA, ptB):
                    vA = vt[kc][:, hA, :]
                    vB = vt[kc][:, hB, :]
                    st, sp = (kc == 0), (kc == 15)
                    nc.tensor.matmul(ot[0], lhsT=vA, rhs=ptA[:, 0:512],
                                     start=st, stop=sp, skip_group_check=True)
                    nc.tensor.matmul(ot[1], lhsT=vA, rhs=ptA[:, 512:1024],
                                     start=st, stop=sp, skip_group_check=True)
                    nc.tensor.matmul(ot[2], lhsT=vB, rhs=ptB[:, 0:512],
                                     start=st, stop=sp, skip_group_check=True)
                    nc.tensor.matmul(ot[3], lhsT=vB, rhs=ptB[:, 512:1024],
                                     start=st, stop=sp, skip_group_check=True)

                pt_hist = []
                for kc in range(16):
                    kA = qkT[kch][0:64, kc * 128:(kc + 1) * 128]
                    kB = qkT[kch][64:128, kc * 128:(kc + 1) * 128]
                    stqA = mm.tile([128, 1024], F32, name="stq", tag="stq")
                    nc.tensor.matmul(
                        stqA[:, 0:512], lhsT=kA,
                        rhs=qkT[qch][0:64, qh * 1024:qh * 1024 + 512],
                        start=True, stop=True)
                    nc.tensor.matmul(
                        stqA[:, 512:1024], lhsT=kA,
                        rhs=qkT[qch][0:64, qh * 1024 + 512:(qh + 1) * 1024],
                        start=True, stop=True)
                    stqB = mm.tile([128, 1024], F32, name="stq", tag="stq")
                    nc.tensor.matmul(
                        stqB[:, 0:512], lhsT=kB,
                        rhs=qkT[qch][64:128, qh * 1024:qh * 1024 + 512],
                        start=True, stop=True)
                    nc.tensor.matmul(
                        stqB[:, 512:1024], lhsT=kB,
                        rhs=qkT[qch][64:128, qh * 1024 + 512:(qh + 1) * 1024],
                        start=True, stop=True)
                    ptA = ptp.tile([128, 1024], F16, name="pt", tag="pt")
                    nc.scalar.activation(ptA, stqA, EXP, scale=SCALE)
                    ptB = ptp.tile([128, 1024], F16, name="pt", tag="pt")
                    nc.scalar.activation(ptB, stqB, EXP, scale=SCALE)
                    pt_hist.append((kc, ptA, ptB))
                    if len(pt_hist) > 2:
                        k2, a2, b2 = pt_hist.pop(0)
                        emit_ot(k2, a2, b2)
                for k2, a2, b2 in pt_hist:
                    emit_ot(k2, a2, b2)
                # normalize (off critical path): raw copies FIRST so all
                # ot banks free quickly; recips/dmas/muls trail behind.
                js = ((0, hA), (1, hA), (2, hB), (3, hB))
                raws, rcs, bcs, s32s = {}, {}, {}, {}
                for j, hh in js:
                    raw = rawp.tile([65, 512], F16, name="raw", tag="raw")
                    nc.vector.tensor_copy(raw, ot[j])
                    raws[j] = raw
                    s32 = tiny.tile([65, 512], F32, name="s32", tag="s32",
                                    bufs=4)
                    nc.vector.tensor_copy(s32[64:65, :], ot[j][64:65, :])
                    s32s[j] = s32
                for j, hh in js:
                    rc = tiny.tile([65, 512], F32, name="rc", tag="rc",
                                   bufs=4)
                    nc.vector.reciprocal(rc[64:65, :], s32s[j][64:65, :])
                    rcs[j] = rc
                for j, hh in js:
                    dsc = dramp.tile([512], F32, name="dsc", tag="dsc")
                    nc.sync.dma_start(out=dsc, in_=rcs[j][64:65, :])
                    bc = tiny.tile([64, 512], F32, name="bc", tag="bc")
                    dap = dsc[:]
                    nc.sync.dma_start(
                        out=bc,
                        in_=bass.AP(tensor=dap.tensor, offset=dap.offset,
                                    ap=[[0, 64]] + list(dap.ap)))
                    bcs[j] = bc
                for j, hh in js:
                    qc = 2 * qh + (j % 2)
                    nc.vector.tensor_mul(
                        otn[hh][:, qc * 512:(qc + 1) * 512],
                        raws[j][0:64, :], bcs[j])

        # ---- output projection: 32 narrow chains, 6 psum slots ---------
        ptags = ["stq", "stq", "ot0", "ot1", "ot2", "ot3"]
        ppools = [mm, mm, acc, acc, acc, acc]
        ci = 0
        for it in range(16):
            for half in range(2):
                tag = ptags[ci % 6]
                ps = ppools[ci % 6].tile([128, 512], F32,
                                         name=f"pj{ci}", tag=tag)
                ci += 1
                e0 = half * 512
                for h in range(HP):
                    nc.tensor.matmul(
                        ps, lhsT=otn[h][:, it * 128:(it + 1) * 128],
                        rhs=wo[h][:, e0:e0 + 512],
                        start=(h == 0), stop=(h == 7))
                yt = ypool.tile([128, 512], F32, name="yt", tag="yt",
                                bufs=4)
                nc.vector.tensor_add(yt, ps, bias[:, e0:e0 + 512])
                nc.sync.dma_start(
                    out=out_d[it * 128:(it + 1) * 128, e0:e0 + 512], in_=yt)

    nc.compile()
    return nc


def _in_maps(x, w_qkv, w_out, b_out):
    x = np.asarray(x, dtype=np.float32)
    w_qkv = np.asarray(w_qkv, dtype=np.float32)
    w_out = np.asarray(w_out, dtype=np.float32)
    b_out = np.asarray(b_out, dtype=np.float32)
    maps = []
    for c in range(NCORES):
        b, g = c // 2, c % 2
        qcols = w_qkv[:, g * GDIM:(g + 1) * GDIM]
        kcols = w_qkv[:, D + g * GDIM:D + (g + 1) * GDIM]
        vcols = w_qkv[:, 2 * D + g * GDIM:2 * D + (g + 1) * GDIM]
        maps.append({
            "xT": np.ascontiguousarray(x[b].T).astype(np.float16),
            "wqk": np.concatenate([qcols, kcols], axis=1).astype(np.float16),
            "wv": np.ascontiguousarray(vcols).astype(np.float16),
            "wo": np.ascontiguousarray(
                w_out[g * GDIM:(g + 1) * GDIM, :].reshape(HP, DH, D)
            ).astype(np.float16),
            "bias": (b_out if g == 0 else np.zeros_like(b_out)),
        })
    return maps


def kernel(x, w_qkv, w_out, b_out):
    from concourse.bass_utils import run_bass_kernel_spmd

    if "nc" not in _CACHE:
        _CACHE["nc"] = _build()
    nc = _CACHE["nc"]
    maps = _in_maps(x, w_qkv, w_out, b_out)
    res = run_bass_kernel_spmd(nc, maps, core_ids=list(range(NCORES)))
    outs = res.results
    y = np.empty((B, N, D), dtype=np.float32)
    for b in range(B):
        y[b] = outs[2 * b]["out"] + outs[2 * b + 1]["out"]
    return y


# revision 19
# speedup vs baseline: 1.6936x; 1.4397x over previous
"""Multi-head attention kernel for 8 TRN2 NeuronCores.

Problem: x[4,2048,1024] -> qkv proj (w_qkv[1024,3072]) -> 16-head attention
(dim_head=64, scale=1024**-0.5) -> out proj (w_out[1024,1024] + b_out).

Sharding: core c in 0..7 handles batch b=c//2, head-group g=c%2 (8 heads).
Each core computes a partial output y_partial = attn_out_g @ w_out[rows_g];
host sums the pair (the tensor-parallel all-reduce, done at unshard time).

Layout strategy (zero on-chip transposes):
  - host supplies xT = x[b].T                     [1024, 2048] fp16
  - qkT = (x @ w_qk).T computed directly:  lhsT=w chunk, rhs=xT  -> [c, i]
  - V   = x @ w_v computed normally:       lhsT=xT chunk, rhs=wv -> [i, c]
  - S^T = k_h @ q_h^T per head:            lhsT=kT slice, rhs=qT slice
          -> [keys, q]; heads processed in pairs, the even head in array
          row-group 0-63 and the odd head in 64-127, so their K=64
          matmuls run concurrently in the PE array
  - P   = exp(S^T * scale)  (no max subtraction: |S*scale| < ~1)
  - O^T|s = [v_h | 1]^T @ P : lhsT=v[128,65] (ones col), rhs=P -> [65, q]
          row 64 is the softmax denominator s
  - normalize off critical path: 1/s row bounced through DRAM and
    broadcast-DMA'd across partitions (no PSUM, no PE involvement)
  - y = sum_h (O_h^T).T @ w_out_h : lhsT=otn[64,128], rhs=wo -> [i, e]
qkv matmuls for later head-pairs are interleaved one-per-kc into the
attention loop as PE filler under the ScalarE-exp-bound steady state.
All matmul inputs fp16, PSUM accumulation fp32, output fp32.
"""

import numpy as np

B, N, D = 4, 2048, 1024
HEADS, DH = 16, 64
HP = HEADS // 2          # heads per core
GDIM = HP * DH           # 512 columns per head-group
SCALE = float(D) ** -0.5
NCORES = 8

_CACHE = {}


def _build():
    from contextlib import ExitStack

    import concourse.bass as bass
    import concourse.tile as tile
    from concourse import bacc, mybir

    F16 = mybir.dt.float16
    F32 = mybir.dt.float32
    EXP = mybir.ActivationFunctionType.Exp
    LN = mybir.ActivationFunctionType.Ln

    nc = bacc.Bacc(None, target_bir_lowering=False)

    xT_d = nc.declare_dram_parameter("xT", [D, N], F16, isOutput=False)
    wqk_d = nc.declare_dram_parameter("wqk", [D, 2 * GDIM], F16, isOutput=False)
    wv_d = nc.declare_dram_parameter("wv", [D, GDIM], F16, isOutput=False)
    wo_d = nc.declare_dram_parameter("wo", [HP, DH, D], F16, isOutput=False)
    bias_d = nc.declare_dram_parameter("bias", [D], F32, isOutput=False)
    out_d = nc.declare_dram_parameter("out", [N, D], F32, isOutput=True)

    with tile.TileContext(nc) as tc, ExitStack() as ctx:
        persist = ctx.enter_context(tc.tile_pool(name="persist", bufs=1))
        ptp = ctx.enter_context(tc.tile_pool(name="ptp", bufs=6))
        rawp = ctx.enter_context(tc.tile_pool(name="rawp", bufs=5))
        tiny = ctx.enter_context(tc.tile_pool(name="tiny", bufs=4))
        ypool = ctx.enter_context(tc.tile_pool(name="ypool", bufs=2))
        dramp = ctx.enter_context(tc.tile_pool(name="dramp", bufs=4,
                                               space="DRAM"))
        # PSUM budget (8 banks): stq [128,1024] x2 bufs = 4, ot x4 = 4... see
        # tags: "stq" 2-bank tiles bufs=2, "ot0..3" 1 bank each, "qf" 1 bank
        mm = ctx.enter_context(tc.tile_pool(name="mm", bufs=2, space="PSUM"))
        acc = ctx.enter_context(tc.tile_pool(name="acc", bufs=1, space="PSUM"))

        # ---- persistent SBUF tiles -------------------------------------
        xT = [persist.tile([128, N], F16, name=f"xT{e}", tag=f"xT{e}")
              for e in range(8)]
        wqk = [persist.tile([128, 2 * GDIM], F16, name=f"wqk{e}", tag=f"wqk{e}")
               for e in range(8)]
        wv = [persist.tile([128, GDIM], F16, name=f"wv{e}", tag=f"wv{e}")
              for e in range(8)]
        wo = [persist.tile([DH, D], F16, name=f"wo{h}", tag=f"wo{h}")
              for h in range(HP)]
        bias = persist.tile([128, D], F32, tag="bias")
        qkT = [persist.tile([128, N], F16, name=f"qkT{c}", tag=f"qkT{c}")
               for c in range(8)]
        vt = [persist.tile([128, HP, DH + 1], F16, name=f"v{kc}", tag=f"v{kc}")
              for kc in range(16)]
        otn = [persist.tile([DH, N], F16, name=f"otn{h}", tag=f"otn{h}")
               for h in range(HP)]

        for e in range(8):
            nc.sync.dma_start(out=xT[e], in_=xT_d[e * 128:(e + 1) * 128, :])
            nc.sync.dma_start(out=wqk[e], in_=wqk_d[e * 128:(e + 1) * 128, :])
            nc.sync.dma_start(out=wv[e], in_=wv_d[e * 128:(e + 1) * 128, :])
        for h in range(HP):
            nc.sync.dma_start(out=wo[h], in_=wo_d[h])
        bias_ap = bias_d[:]
        nc.sync.dma_start(
            out=bias,
            in_=bass.AP(tensor=bias_ap.tensor, offset=bias_ap.offset,
                        ap=[[0, 128]] + list(bias_ap.ap)),
        )
        for kc in range(16):
            nc.vector.memset(vt[kc][:, :, DH:DH + 1], 1.0)

        def qkv_chain(c, ih):
            """One [128,1024] qkT chunk: chunk c, query half ih."""
            ps = mm.tile([128, 1024], F32, name="stq", tag="stq")
            for e in range(8):
                w_sl = wqk[e][:, c * 128:(c + 1) * 128]
                yield nc.tensor.matmul(
                    ps[:, 0:512], lhsT=w_sl,
                    rhs=xT[e][:, ih * 1024:ih * 1024 + 512],
                    start=(e == 0), stop=(e == 7))
                yield nc.tensor.matmul(
                    ps[:, 512:1024], lhsT=w_sl,
                    rhs=xT[e][:, ih * 1024 + 512:(ih + 1) * 1024],
                    start=(e == 0), stop=(e == 7))
            yield nc.vector.tensor_copy(
                qkT[c][:, ih * 1024:(ih + 1) * 1024], ps)

        def v_chain(ih):
            """Two key-tiles of V via one [128,1024] psum tile."""
            ps = mm.tile([128, 1024], F32, name="stq", tag="stq")
            for e in range(8):
                yield nc.tensor.matmul(
                    ps[:, 0:512],
                    lhsT=xT[e][:, (2 * ih) * 128:(2 * ih + 1) * 128],
                    rhs=wv[e], start=(e == 0), stop=(e == 7))
                yield nc.tensor.matmul(
                    ps[:, 512:1024],
                    lhsT=xT[e][:, (2 * ih + 1) * 128:(2 * ih + 2) * 128],
                    rhs=wv[e], start=(e == 0), stop=(e == 7))
            for j in range(2):
                yield nc.vector.tensor_copy(
                    vt[2 * ih + j][:, :, 0:DH],
                    ps[:, j * 512:(j + 1) * 512].rearrange(
                        "p (h d) -> p h d", h=HP))

        # ---- PE warm-up: dummy matmuls during the input-DMA window ------
        wu = persist.tile([128, 512], F16, tag="wu")
        nc.vector.memset(wu, 0.0)
        wps = mm.tile([128, 1024], F32, name="stq", tag="stq")
        for r in range(48):
            nc.tensor.matmul(wps[:, 0:512], lhsT=wu[:, 0:128], rhs=wu,
                             start=True, stop=True)

        # ---- prelude: V first, then qkT ordered so pair-0 chunks land
        # last (dense PE hand-off into the first attention pass) ----------
        for ih in range(8):
            for g in v_chain(ih):
                pass
        for c in (3, 7, 2, 6, 1, 5, 0, 4):
            for ih in range(2):
                for g in qkv_chain(c, ih):
                    pass

        # ---- attention: head pairs, packed ST row-groups ---------------
        for t in range(4):
            hA, hB = 2 * t, 2 * t + 1
            qch, kch = t, 4 + t
            for qh in range(2):
                ot = {}
                for j, hh in ((0, hA), (1, hA), (2, hB), (3, hB)):
                    ot[j] = acc.tile([65, 512], F32,
                                     name=f"ot{t}_{qh}_{j}", tag=f"ot{j}")
                def emit_ot(kc, ptA, ptB):
                    vA = vt[kc][:, hA, :]
                    vB = vt[kc][:, hB, :]
                    st, sp = (kc == 0), (kc == 15)
                    nc.tensor.matmul(ot[0], lhsT=vA, rhs=ptA[:, 0:512],
                                     start=st, stop=sp, skip_group_check=True)
                    nc.tensor.matmul(ot[1], lhsT=vA, rhs=ptA[:, 512:1024],
                                     start=st, stop=sp, skip_group_check=True)
                    nc.tensor.matmul(ot[2], lhsT=vB, rhs=ptB[:, 0:512],
                                     start=st, stop=sp, skip_group_check=True)
                    nc.tensor.matmul(ot[3], lhsT=vB, rhs=ptB[:, 512:1024],
                                     start=st, stop=sp, skip_group_check=True)

                pt_hist = []
                for kc in range(16):
                    kA = qkT[kch][0:64, kc * 128:(kc + 1) * 128]
                    kB = qkT[kch][64:128, kc * 128:(kc + 1) * 128]
                    stqA = mm.tile([128, 1024], F32, name="stq", tag="stq")
                    nc.tensor.matmul(
                        stqA[:, 0:512], lhsT=kA,
                        rhs=qkT[qch][0:64, qh * 1024:qh * 1024 + 512],
                        start=True, stop=True)
                    nc.tensor.matmul(
                        stqA[:, 512:1024], lhsT=kA,
                        rhs=qkT[qch][0:64, qh * 1024 + 512:(qh + 1) * 1024],
                        start=True, stop=True)
                    stqB = mm.tile([128, 1024], F32, name="stq", tag="stq")
                    nc.tensor.matmul(
                        stqB[:, 0:512], lhsT=kB,
                        rhs=qkT[qch][64:128, qh * 1024:qh * 1024 + 512],
                        start=True, stop=True)
                    nc.tensor.matmul(
                        stqB[:, 512:1024], lhsT=kB,
                        rhs=qkT[qch][64:128, qh * 1024 + 512:(qh + 1) * 1024],
                        start=True, stop=True)
                    ptA = ptp.tile([128, 1024], F16, name="pt", tag="pt")
                    nc.scalar.activation(ptA, stqA, EXP, scale=SCALE)
                    ptB = ptp.tile([128, 1024], F16, name="pt", tag="pt")
                    nc.scalar.activation(ptB, stqB, EXP, scale=SCALE)
                    pt_hist.append((kc, ptA, ptB))
                    if len(pt_hist) > 2:
                        k2, a2, b2 = pt_hist.pop(0)
                        emit_ot(k2, a2, b2)
                for k2, a2, b2 in pt_hist:
                    emit_ot(k2, a2, b2)
                # normalize (off critical path): raw copies FIRST so all
                # ot banks free quickly; recips/dmas/muls trail behind.
                js = ((0, hA), (1, hA), (2, hB), (3, hB))
                raws, rcs, bcs, s32s = {}, {}, {}, {}
                for j, hh in js:
                    raw = rawp.tile([65, 512], F16, name="raw", tag="raw")
                    nc.vector.tensor_copy(raw, ot[j])
                    raws[j] = raw
                    s32 = tiny.tile([65, 512], F32, name="s32", tag="s32",
                                    bufs=4)
                    nc.vector.tensor_copy(s32[64:65, :], ot[j][64:65, :])
                    s32s[j] = s32
                last_pass = (t == 3 and qh == 1)
                for j, hh in js:
                    rc = tiny.tile([65, 512], F32, name="rc", tag="rc",
                                   bufs=4)
                    if last_pass:
                        # ACT is idle after the final exp; 1/s = exp(-ln(s))
                        # is ~5x lower latency than the DVE iterative divide
                        lntmp = tiny.tile([65, 512], F32, name="lntmp",
                                          tag="lntmp", bufs=2)
                        nc.scalar.activation(lntmp[64:65, :],
                                             s32s[j][64:65, :], LN)
                        nc.scalar.activation(rc[64:65, :], lntmp[64:65, :],
                                             EXP, scale=-1.0)
                    else:
                        nc.vector.reciprocal(rc[64:65, :], s32s[j][64:65, :])
                    rcs[j] = rc
                for j, hh in js:
                    dsc = dramp.tile([512], F32, name="dsc", tag="dsc")
                    nc.sync.dma_start(out=dsc, in_=rcs[j][64:65, :])
                    bc = tiny.tile([64, 512], F32, name="bc", tag="bc")
                    dap = dsc[:]
                    nc.sync.dma_start(
                        out=bc,
                        in_=bass.AP(tensor=dap.tensor, offset=dap.offset,
                                    ap=[[0, 64]] + list(dap.ap)))
                    bcs[j] = bc
                for j, hh in js:
                    qc = 2 * qh + (j % 2)
                    nc.vector.tensor_mul(
                        otn[hh][:, qc * 512:(qc + 1) * 512],
                        raws[j][0:64, :], bcs[j])

        # ---- output projection: 32 narrow chains, 6 psum slots ---------
        ptags = ["stq", "stq", "ot0", "ot1", "ot2", "ot3"]
        ppools = [mm, mm, acc, acc, acc, acc]
        ci = 0
        for it in range(16):
            for half in range(2):
                tag = ptags[ci % 6]
                ps = ppools[ci % 6].tile([128, 512], F32,
                                         name=f"pj{ci}", tag=tag)
                ci += 1
                e0 = half * 512
                for h in range(HP):
                    nc.tensor.matmul(
                        ps, lhsT=otn[h][:, it * 128:(it + 1) * 128],
                        rhs=wo[h][:, e0:e0 + 512],
                        start=(h == 0), stop=(h == 7))
                yt = ypool.tile([128, 512], F32, name="yt", tag="yt",
                                bufs=2)
                nc.vector.tensor_add(yt, ps, bias[:, e0:e0 + 512])
                nc.sync.dma_start(
                    out=out_d[it * 128:(it + 1) * 128, e0:e0 + 512], in_=yt)

    nc.compile()
    return nc


def _in_maps(x, w_qkv, w_out, b_out):
    x = np.asarray(x, dtype=np.float32)
    w_qkv = np.asarray(w_qkv, dtype=np.float32)
    w_out = np.asarray(w_out, dtype=np.float32)
    b_out = np.asarray(b_out, dtype=np.float32)
    maps = []
    for c in range(NCORES):
        b, g = c // 2, c % 2
        qcols = w_qkv[:, g * GDIM:(g + 1) * GDIM]
        kcols = w_qkv[:, D + g * GDIM:D + (g + 1) * GDIM]
        vcols = w_qkv[:, 2 * D + g * GDIM:2 * D + (g + 1) * GDIM]
        maps.append({
            "xT": np.ascontiguousarray(x[b].T).astype(np.float16),
            "wqk": np.concatenate([qcols, kcols], axis=1).astype(np.float16),
            "wv": np.ascontiguousarray(vcols).astype(np.float16),
            "wo": np.ascontiguousarray(
                w_out[g * GDIM:(g + 1) * GDIM, :].reshape(HP, DH, D)
            ).astype(np.float16),
            "bias": (b_out if g == 0 else np.zeros_like(b_out)),
        })
    return maps


def kernel(x, w_qkv, w_out, b_out):
    from concourse.bass_utils import run_bass_kernel_spmd

    if "nc" not in _CACHE:
        _CACHE["nc"] = _build()
    nc = _CACHE["nc"]
    maps = _in_maps(x, w_qkv, w_out, b_out)
    res = run_bass_kernel_spmd(nc, maps, core_ids=list(range(NCORES)))
    outs = res.results
    y = np.empty((B, N, D), dtype=np.float32)
    for b in range(B):
        y[b] = outs[2 * b]["out"] + outs[2 * b + 1]["out"]
    return y
